# revision 27
# baseline (speedup 1.0000x reference)
"""Trainium2 Bass kernel for nn_Encoder_77627238908751.

Strategy (8 NeuronCores, SPMD single program, role differences carried
entirely by per-core input DATA — no rank branching):

- Phase B (all cores): each core gathers 1/8 of the src embeddings for BOTH
  sequence orders (forward s-ascending and backward s-descending), PE-transposes
  them to feature-major, computes its 1/8 slice of the GRU input-gate
  preactivations gx^T = Wih @ emb^T for both directions, and AllGathers the
  full gx^T tensors so cores 0/1 have their direction's gx locally.
- Phase C: the sequential GRU recurrence, fully unrolled.  Core 0 runs the
  forward direction, core 1 the backward one (its inputs are the reversed
  sequence, so the same program computes the backward pass); helper cores run
  the same instructions on zero weights.  Feature-major layout: per step
  48 (128x128)x(128x32) bf16 matmuls accumulate the 3 gate preactivations in
  PSUM, gate math on DVE/ACT in fp32, h kept as an fp32 master + bf16 stream
  copy.  Each step's h is PE-transposed and written (masked per core) into
  per-window DRAM buffers; each 32-step window is ReduceScattered across the
  8 cores during the recurrence, leaving every core exactly its 4-batch shard
  of the bidirectional outputs.
- Phase D (all cores): additive-attention + output projection for the core's
  4 batches.  Key algebraic simplifications: scores @ theme_proj is rank-1
  (enc_w_2[b,s,:] = (sum_t scores[b,s,t]) * theme_proj[b,:]), so the (B,S,T,H)
  tensor is never materialized — tanh tiles are reduced on the fly against wv
  in PSUM, and the enc_w_2 @ Wo1^T term becomes a rank-1 outer-product matmul
  fused into the Wo2 accumulation.

Host side only reorders/slices/casts inputs and reassembles outputs.
"""

import sys
import types

sys.path.insert(0, "/opt/trn_rl_repo")
sys.path.insert(0, "/root/.axon_site")

import numpy as np
import ml_dtypes
import orjson

import concourse.bass as bass
import concourse.tile as tile
import concourse.mybir as mybir
from concourse.bass_utils import run_bass_kernel_spmd
from concourse.masks import make_identity

dt = mybir.dt
AF = mybir.ActivationFunctionType
ALU = mybir.AluOpType

# ---------------------------------------------------------------------------
# workaround: this walrus build rejects >1 sync wait per instruction; split
# extra on_wait entries onto preceding single-wait EventSemaphore instructions.
# ---------------------------------------------------------------------------
_MAX_WAITS = 1


def _split_waits(mod: dict) -> bool:
    changed = False
    ctr = 0
    for f in mod.get("functions", []):
        for bb in f.get("blocks", []):
            out = []
            for ins in bb.get("instructions", []):
                si = ins.get("sync_info")
                waits = (si or {}).get("on_wait") or []
                if len(waits) > _MAX_WAITS:
                    changed = True
                    extra, keep = waits[:-_MAX_WAITS], waits[-_MAX_WAITS:]
                    for w in extra:
                        ctr += 1
                        out.append({
                            "debug": ins.get("debug", 0),
                            "engine": ins["engine"],
                            "ins": [],
                            "name": f"{ins['name']}_xw{ctr}",
                            "opcode": "EventSemaphore",
                            "outs": [],
                            "sync_info": {"on_update": [], "on_wait": [w]},
                        })
                    si["on_wait"] = keep
                out.append(ins)
            bb["instructions"] = out
    return changed


if not getattr(bass.Bass, "_waitfix_installed", False):
    _orig_to_json_bytes = bass.Bass.to_json_bytes

    def _patched_to_json_bytes(self) -> bytes:
        raw = _orig_to_json_bytes(self)
        mod = orjson.loads(raw)
        if _split_waits(mod):
            raw = orjson.dumps(mod)
        return raw

    bass.Bass.to_json_bytes = _patched_to_json_bytes
    bass.Bass._waitfix_installed = True

# ---------------------------------------------------------------------------
# problem constants
# ---------------------------------------------------------------------------
V, H, B, S, T = 32000, 512, 32, 256, 16
NC_ = 8            # cores
C = 4              # 128-feature chunks of H
G3 = 3 * H         # 1536
WS = 32            # ReduceScatter window (steps)
NW = S // WS       # windows
BL = B // NC_      # batches per core (4)
TLOC = S * B // NC_  # tokens per core per direction (1024)
NT = TLOC // 128     # token tiles per core (8)
NB = TLOC // 512     # 512-token blocks per core (2)
PW = 16            # gx prefetch half-window (steps)


def _build_nc(v=V, s_steps=S):
    """Build the single SPMD Bass program."""
    nw = s_steps // WS
    tloc = s_steps * B // NC_
    nt = max(tloc // 128, 1)
    nb = max(tloc // 512, 1)
    nbl = min(512, tloc)           # token block width for gx matmul
    pw = min(PW, WS, tloc // B)

    nc = bass.Bass()

    # ---- I/O ------------------------------------------------------------
    src_tab = nc.declare_dram_parameter("src_tab", [v, H], dt.float32, isOutput=False)
    theme_tab = nc.declare_dram_parameter("theme_tab", [v, H], dt.float32, isOutput=False)
    keyword_tab = nc.declare_dram_parameter("keyword_tab", [v, H], dt.float32, isOutput=False)
    ids_f = nc.declare_dram_parameter("ids_f", [128, nt], dt.int32, isOutput=False)
    ids_b = nc.declare_dram_parameter("ids_b", [128, nt], dt.int32, isOutput=False)
    wihT_f = nc.declare_dram_parameter("wihT_f", [H, G3], dt.bfloat16, isOutput=False)
    wihT_b = nc.declare_dram_parameter("wihT_b", [H, G3], dt.bfloat16, isOutput=False)
    whhT = nc.declare_dram_parameter("whhT", [H, G3], dt.bfloat16, isOutput=False)
    gxbias_f = nc.declare_dram_parameter("gxbias_f", [128, 12], dt.float32, isOutput=False)
    gxbias_b = nc.declare_dram_parameter("gxbias_b", [128, 12], dt.float32, isOutput=False)
    bbnb = nc.declare_dram_parameter("bbnb", [128, 128], dt.float32, isOutput=False)
    mF = nc.declare_dram_parameter("mF", [128, 1], dt.float32, isOutput=False)
    mB = nc.declare_dram_parameter("mB", [128, 1], dt.float32, isOutput=False)
    theme_ids = nc.declare_dram_parameter("theme_ids", [BL, 1], dt.int32, isOutput=False)
    kw_ids = nc.declare_dram_parameter("kw_ids", [BL * T, 1], dt.int32, isOutput=False)
    wwT = nc.declare_dram_parameter("wwT", [2 * H, H], dt.bfloat16, isOutput=False)
    wo1T = nc.declare_dram_parameter("wo1T", [H, H], dt.bfloat16, isOutput=False)
    wo2T = nc.declare_dram_parameter("wo2T", [2 * H, H], dt.bfloat16, isOutput=False)
    wtT = nc.declare_dram_parameter("wtT", [H, H], dt.bfloat16, isOutput=False)
    wkT = nc.declare_dram_parameter("wkT", [H, H], dt.bfloat16, isOutput=False)
    whoT = nc.declare_dram_parameter("whoT", [2 * H, H], dt.bfloat16, isOutput=False)
    wv_c = nc.declare_dram_parameter("wv_c", [128, C], dt.bfloat16, isOutput=False)
    btT = nc.declare_dram_parameter("btT", [128, C], dt.float32, isOutput=False)
    bkwT = nc.declare_dram_parameter("bkwT", [128, C], dt.float32, isOutput=False)
    boT = nc.declare_dram_parameter("boT", [128, C], dt.float32, isOutput=False)
    tbv = nc.declare_dram_parameter("tbv", [128, 1], dt.float32, isOutput=False)

    enc_outT = nc.declare_dram_parameter("enc_outT", [BL, C, 128, s_steps], dt.float32, isOutput=True)
    hidT = nc.declare_dram_parameter("hidT", [BL, C, 128], dt.float32, isOutput=True)

    rg = [list(range(NC_))]

    from contextlib import ExitStack

    with tile.TileContext(nc) as tc, ExitStack() as stack:
        # persistent pools
        const = stack.enter_context(tc.tile_pool(name="const", bufs=1))
        dram = stack.enter_context(tc.tile_pool(name="dram", bufs=1, space="DRAM"))

        ident = const.tile([128, 128], dt.float32)
        make_identity(nc, ident[:])
        identBL = const.tile([BL, BL], dt.float32)
        make_identity(nc, identBL[:])
        identKW = const.tile([BL * T, BL * T], dt.float32)
        make_identity(nc, identKW[:])
        identWS = const.tile([WS, WS], dt.bfloat16)
        make_identity(nc, identWS[:])

        mF_sb = const.tile([128, 1], dt.float32)
        nc.sync.dma_start(mF_sb[:], mF[:])
        mB_sb = const.tile([128, 1], dt.float32)
        nc.sync.dma_start(mB_sb[:], mB[:])
        identBF = const.tile([128, 128], dt.bfloat16)
        nc.vector.tensor_copy(identBF[:], ident[:])
        identF = const.tile([128, 128], dt.bfloat16)
        nc.vector.tensor_scalar_mul(identF[:], ident[:], mF_sb[:])
        identB = const.tile([128, 128], dt.bfloat16)
        nc.vector.tensor_scalar_mul(identB[:], ident[:], mB_sb[:])
        bbnb_sb = const.tile([128, 128], dt.float32)
        nc.sync.dma_start(bbnb_sb[:], bbnb[:])
        bbnb_bf = const.tile([128, 128], dt.bfloat16)
        nc.vector.tensor_copy(bbnb_bf[:], bbnb_sb[:])

        whh_sb = const.tile([128, C, G3], dt.bfloat16)
        nc.sync.dma_start(whh_sb[:], whhT[:].rearrange("(k p) g -> p k g", p=128))

        # DRAM intermediates
        gx_contrib = {}
        gx_full = {}
        for d in ("f", "b"):
            gx_contrib[d] = dram.tile([128, 12, tloc], dt.bfloat16, name=f"gxc{d}")
            gx_full[d] = dram.tile([NC_, 128, 12, tloc], dt.bfloat16, name=f"gxfull{d}", addr_space="Shared")
        bufF = [dram.tile([B, C, WS, 128], dt.bfloat16, name=f"bufF{w}") for w in range(nw)]
        bufB = [dram.tile([B, C, WS, 128], dt.bfloat16, name=f"bufB{w}") for w in range(nw)]
        rsF = [dram.tile([BL, C, WS, 128], dt.bfloat16, name=f"rsF{w}") for w in range(nw)]
        rsB = [dram.tile([BL, C, WS, 128], dt.bfloat16, name=f"rsB{w}") for w in range(nw)]

        # ---- Phase B: embedding gather + transpose + gx slices + AllGather
        _scopeB = nc.named_scope("phaseB"); _scopeB.__enter__()
        with (
            tc.tile_pool(name="pb_sb", bufs=2) as pb,
            tc.tile_pool(name="pb_semb", bufs=1) as pb_semb,
            tc.tile_pool(name="pb_wih", bufs=1) as pb_wih,
            tc.tile_pool(name="pb_ps", bufs=2, space="PSUM") as pb_ps,
        ):
            wih_sb = {}
            for d, w_in in (("f", wihT_f), ("b", wihT_b)):
                w_sb = pb_wih.tile([128, C, G3], dt.bfloat16, name=f"wih{d}", tag=f"wih{d}")
                nc.sync.dma_start(w_sb[:], w_in[:].rearrange("(k p) g -> p k g", p=128))
                wih_sb[d] = w_sb
            gxb_sb = {}
            for d, b_in in (("f", gxbias_f), ("b", gxbias_b)):
                t_ = const.tile([128, 12], dt.float32, name=f"gxb{d}", tag=f"gxb{d}")
                nc.sync.dma_start(t_[:], b_in[:])
                gxb_sb[d] = t_

            idt = {}
            for d, i_in in (("f", ids_f), ("b", ids_b)):
                t_ = const.tile([128, nt], dt.int32, name=f"idt{d}", tag=f"idt{d}")
                nc.sync.dma_start(t_[:], i_in[:])
                idt[d] = t_

            semb = {}
            for d in ("f", "b"):
                semb[d] = [pb_semb.tile([128, nt * 128], dt.bfloat16, name=f"semb{d}{c}", tag=f"semb{d}{c}")
                           for c in range(C)]
                for tt in range(nt):
                    g = pb.tile([128, H], dt.float32, tag="gath")
                    nc.gpsimd.indirect_dma_start(
                        out=g[:], out_offset=None, in_=src_tab[:, :],
                        in_offset=bass.IndirectOffsetOnAxis(ap=idt[d][:, tt:tt + 1], axis=0),
                    )
                    ps = pb_ps.tile([128, H], dt.float32, tag="tp")
                    for c in range(C):
                        nc.tensor.transpose(
                            out=ps[:, c * 128:(c + 1) * 128],
                            in_=g[:, c * 128:(c + 1) * 128],
                            identity=ident[:],
                        )
                    for c in range(C):
                        nc.vector.tensor_copy(
                            semb[d][c][:, tt * 128:(tt + 1) * 128],
                            ps[:, c * 128:(c + 1) * 128],
                        )

            for d in ("f", "b"):
                for m in range(12):
                    for q in range(tloc // nbl):
                        ps = pb_ps.tile([128, nbl], dt.float32, tag="gx")
                        for k in range(C):
                            nc.tensor.matmul(
                                out=ps[:],
                                lhsT=wih_sb[d][:, k, m * 128:(m + 1) * 128],
                                rhs=semb[d][k][:, q * nbl:(q + 1) * nbl],
                                start=(k == 0), stop=(k == C - 1),
                            )
                        gxs = pb.tile([128, nbl], dt.bfloat16, tag="gxs")
                        nc.scalar.activation(gxs[:], ps[:], AF.Identity,
                                             bias=gxb_sb[d][:, m:m + 1])
                        nc.sync.dma_start(
                            gx_contrib[d][:, m, q * nbl:(q + 1) * nbl], gxs[:])

            for d in ("f", "b"):
                nc.gpsimd.collective_compute(
                    "AllGather", ALU.bypass, replica_groups=rg,
                    ins=[gx_contrib[d].opt()], outs=[gx_full[d].opt()],
                )

        _scopeB.__exit__(None, None, None)
        # ---- Phase C: recurrence -----------------------------------------
        _scopeC = nc.named_scope("phaseC"); _scopeC.__enter__()
        with (
            tc.tile_pool(name="pc_h", bufs=2) as pc_h,
            tc.tile_pool(name="pc_g", bufs=2) as pc_g,
            tc.tile_pool(name="pc_win", bufs=2) as pc_win,
            tc.tile_pool(name="pc_ps", bufs=2, space="PSUM") as pc_ps,
            tc.tile_pool(name="pc_pst", bufs=1, space="PSUM") as pc_pst,
        ):
            hbf = pc_h.tile([128, 128], dt.bfloat16, tag="hbf")
            nc.gpsimd.memset(hbf[:], 0.0)

            # per-core AG slot: the recurrence consumes windows by rank-
            # uniform global step index; window w tokens live in slot
            # (w*WS*B)//tloc at offset (w*WS*B) % tloc.
            gxwin = None
            GATE = ((0, "r"), (2, "n"), (1, "z"))  # gate order; m-base = idx*4

            for t in range(s_steps):
                if t % pw == 0:
                    tok0 = t * B
                    slot = tok0 // tloc
                    off = tok0 % tloc
                    wid = pw * B
                    raws = {}
                    for d, eng in (("f", nc.sync), ("b", nc.scalar)):
                        rw = pc_win.tile([128, 12, wid], dt.bfloat16, tag=f"raw{d}")
                        eng.dma_start(rw[:], gx_full[d][slot, :, :, off:off + wid])
                        raws[d] = rw
                    gxwin = pc_win.tile([128, 12, wid], dt.bfloat16, tag="gxwin")
                    nc.vector.tensor_scalar_mul(gxwin[:], raws["b"][:], mB_sb[:])
                    nc.vector.scalar_tensor_tensor(
                        out=gxwin[:], in0=raws["f"][:], scalar=mF_sb[:],
                        in1=gxwin[:], op0=ALU.mult, op1=ALU.add)

                toff = (t % pw) * B
                ps_g = {}
                for gi, gname in GATE:
                    ps = pc_ps.tile([128, 128], dt.float32, tag=f"ps{gname}")
                    for c in range(C):
                        m = gi * 4 + c
                        for k in range(C):
                            nc.tensor.matmul(
                                out=ps[:, c * B:c * B + B],
                                lhsT=whh_sb[:, k, m * 128:(m + 1) * 128],
                                rhs=hbf[:, k * B:k * B + B],
                                start=(k == 0), stop=(k == C - 1),
                            )
                    ps_g[gname] = ps

                def gx_slice(gi):
                    return gxwin[:, gi * 4:(gi + 1) * 4, toff:toff + B]

                def ps3(ps):
                    return ps[:].rearrange("p (c b) -> p c b", c=C)

                rpre = pc_g.tile([128, 128], dt.float32, tag="rpre")
                nc.vector.tensor_add(ps3(rpre), ps3(ps_g["r"]), gx_slice(0))
                r = pc_g.tile([128, 128], dt.float32, tag="r")
                nc.scalar.activation(r[:], rpre[:], AF.Sigmoid)

                npre = pc_g.tile([128, 128], dt.float32, tag="npre")
                nc.vector.tensor_add(npre[:], ps_g["n"][:], bbnb_sb[:])
                nc.vector.tensor_mul(npre[:], r[:], npre[:])
                nc.vector.tensor_add(ps3(npre), ps3(npre), gx_slice(2))
                n = pc_g.tile([128, 128], dt.float32, tag="n")
                nc.scalar.activation(n[:], npre[:], AF.Tanh)

                s1 = pc_g.tile([128, 128], dt.float32, tag="s1")
                nc.vector.tensor_sub(s1[:], hbf[:], n[:])

                zpre = pc_g.tile([128, 128], dt.float32, tag="zpre")
                nc.vector.tensor_add(ps3(zpre), ps3(ps_g["z"]), gx_slice(1))
                z = pc_g.tile([128, 128], dt.float32, tag="z")
                nc.scalar.activation(z[:], zpre[:], AF.Sigmoid)

                tmp = pc_g.tile([128, 128], dt.float32, tag="s2")
                nc.vector.scalar_tensor_tensor(
                    out=tmp[:], in0=z[:], scalar=1.0, in1=s1[:],
                    op0=ALU.bypass, op1=ALU.mult)
                hbfn = pc_h.tile([128, 128], dt.bfloat16, tag="hbf")
                nc.vector.tensor_add(hbfn[:], tmp[:], n[:])
                hbf = hbfn

                # masked transposes straight to PSUM, DMA out from PSUM
                pstF = pc_pst.tile([128, 128], dt.bfloat16, tag="trF")
                nc.tensor.transpose(out=pstF[:], in_=hbf[:], identity=identF[:])
                pstB = pc_pst.tile([128, 128], dt.bfloat16, tag="trB")
                nc.tensor.transpose(out=pstB[:], in_=hbf[:], identity=identB[:])

                stF = pc_g.tile([128, 128], dt.bfloat16, tag="stF")
                nc.vector.tensor_copy(stF[:], pstF[:])
                stB = pc_g.tile([128, 128], dt.bfloat16, tag="stB")
                nc.vector.tensor_copy(stB[:], pstB[:])
                w = t // WS
                tl = t - w * WS
                dstF = bufF[w][:, :, tl, :].rearrange("b c f -> c b f")
                nc.sync.dma_start(dstF, stF[:])
                tlb = WS - 1 - tl
                dstB = bufB[w][:, :, tlb, :].rearrange("b c f -> c b f")
                nc.sync.dma_start(dstB, stB[:])

                if tl == WS - 1:
                    nc.gpsimd.collective_compute(
                        "ReduceScatter", ALU.add, replica_groups=rg,
                        ins=[bufF[w].opt()], outs=[rsF[w].opt()])
                    nc.gpsimd.collective_compute(
                        "ReduceScatter", ALU.add, replica_groups=rg,
                        ins=[bufB[w].opt()], outs=[rsB[w].opt()])

        _scopeC.__exit__(None, None, None)
        # ---- Phase D: attention + outputs --------------------------------
        _scopeD = nc.named_scope("phaseD"); _scopeD.__enter__()
        with (
            tc.tile_pool(name="pd_w", bufs=1) as pd_w,
            tc.tile_pool(name="pd_sb", bufs=2) as pd,
            tc.tile_pool(name="pd_ew", bufs=2) as pd_ew,
            tc.tile_pool(name="pd_ps", bufs=1, space="PSUM") as pd_ps,
            tc.tile_pool(name="pd_ps2", bufs=1, space="PSUM") as pd_ps2,
        ):
            ww_sb = pd_w.tile([128, 8, H], dt.bfloat16)
            nc.sync.dma_start(ww_sb[:], wwT[:].rearrange("(k p) g -> p k g", p=128))
            wo2_sb = pd_w.tile([128, 8, H], dt.bfloat16)
            nc.sync.dma_start(wo2_sb[:], wo2T[:].rearrange("(k p) g -> p k g", p=128))
            wo1_sb = pd_w.tile([128, C, H], dt.bfloat16)
            nc.sync.dma_start(wo1_sb[:], wo1T[:].rearrange("(k p) g -> p k g", p=128))
            wt_sb = pd_w.tile([128, C, H], dt.bfloat16)
            nc.sync.dma_start(wt_sb[:], wtT[:].rearrange("(k p) g -> p k g", p=128))
            wk_sb = pd_w.tile([128, C, H], dt.bfloat16)
            nc.sync.dma_start(wk_sb[:], wkT[:].rearrange("(k p) g -> p k g", p=128))
            who_sb = pd_w.tile([128, 8, H], dt.bfloat16)
            nc.sync.dma_start(who_sb[:], whoT[:].rearrange("(k p) g -> p k g", p=128))
            wv_sb = pd_w.tile([128, C], dt.bfloat16)
            nc.sync.dma_start(wv_sb[:], wv_c[:])
            bt_sb = pd_w.tile([128, C], dt.float32)
            nc.sync.dma_start(bt_sb[:], btT[:])
            bkw_sb = pd_w.tile([128, C], dt.float32)
            nc.sync.dma_start(bkw_sb[:], bkwT[:])
            bo_sb = pd_w.tile([128, C], dt.float32)
            nc.sync.dma_start(bo_sb[:], boT[:])
            tbv_sb = pd_w.tile([128, 1], dt.float32)
            nc.sync.dma_start(tbv_sb[:], tbv[:])

        # theme / keyword embeddings + projections (per-core b-shard)
            tid = pd_w.tile([BL, 1], dt.int32)
            nc.sync.dma_start(tid[:], theme_ids[:])
            kid = pd_w.tile([BL * T, 1], dt.int32)
            nc.sync.dma_start(kid[:], kw_ids[:])
            th_emb = pd_w.tile([BL, H], dt.float32)
            nc.gpsimd.indirect_dma_start(
                out=th_emb[:], out_offset=None, in_=theme_tab[:, :],
                in_offset=bass.IndirectOffsetOnAxis(ap=tid[:, :1], axis=0))
            kw_emb = pd_w.tile([BL * T, H], dt.float32)
            nc.gpsimd.indirect_dma_start(
                out=kw_emb[:], out_offset=None, in_=keyword_tab[:, :],
                in_offset=bass.IndirectOffsetOnAxis(ap=kid[:, :1], axis=0))

            ps_th = pd_ps.tile([128, C * BL], dt.float32, tag="small")
            for c in range(C):
                nc.tensor.transpose(out=ps_th[:, c * BL:(c + 1) * BL],
                                    in_=th_emb[:, c * 128:(c + 1) * 128],
                                    identity=identBL[:])
            thT = pd_w.tile([128, C * BL], dt.bfloat16)
            nc.vector.tensor_copy(thT[:], ps_th[:])

            ps_kw = pd_ps.tile([128, C * BL * T], dt.float32, tag="kw")
            for c in range(C):
                nc.tensor.transpose(out=ps_kw[:, c * BL * T:(c + 1) * BL * T],
                                    in_=kw_emb[:, c * 128:(c + 1) * 128],
                                    identity=identKW[:])
            kwT = pd_w.tile([128, C * BL * T], dt.bfloat16)
            nc.vector.tensor_copy(kwT[:], ps_kw[:])

            # tpT (feature-major theme projection, fp32 + bf16)
            ps_tp = pd_ps.tile([128, C * BL], dt.float32, tag="small")
            for m in range(C):
                for k in range(C):
                    nc.tensor.matmul(
                        out=ps_tp[:, m * BL:(m + 1) * BL],
                        lhsT=wt_sb[:, k, m * 128:(m + 1) * 128],
                        rhs=thT[:, k * BL:(k + 1) * BL],
                        start=(k == 0), stop=(k == C - 1))
            tpT = pd_w.tile([128, C * BL], dt.float32)
            for m in range(C):
                nc.scalar.activation(tpT[:, m * BL:(m + 1) * BL],
                                     ps_tp[:, m * BL:(m + 1) * BL],
                                     AF.Identity, bias=bt_sb[:, m:m + 1])
            tpT_bf = pd_w.tile([128, C * BL], dt.bfloat16)
            nc.vector.tensor_copy(tpT_bf[:], tpT[:])

            # b3T + bias2
            ps_b3 = pd_ps.tile([128, C * BL * T], dt.float32, tag="kw")
            for m in range(C):
                for k in range(C):
                    nc.tensor.matmul(
                        out=ps_b3[:, m * BL * T:(m + 1) * BL * T],
                        lhsT=wk_sb[:, k, m * 128:(m + 1) * 128],
                        rhs=kwT[:, k * BL * T:(k + 1) * BL * T],
                        start=(k == 0), stop=(k == C - 1))
            b3_sb = pd_w.tile([128, C * BL * T], dt.float32)
            for m in range(C):
                nc.scalar.activation(b3_sb[:, m * BL * T:(m + 1) * BL * T],
                                     ps_b3[:, m * BL * T:(m + 1) * BL * T],
                                     AF.Identity, bias=bkw_sb[:, m:m + 1])
            bias2 = pd_w.tile([128, C * BL * T], dt.float32)
            nc.vector.tensor_add(
                bias2[:].rearrange("p (c b t) -> p c b t", c=C, b=BL),
                b3_sb[:].rearrange("p (c b t) -> p c b t", c=C, b=BL),
                tpT[:].rearrange("p (c b) -> p c b", c=C)[:, :, :, None].to_broadcast([128, C, BL, T]),
            )

            # tpo row-major + feature-major + final bias
            ps_tpo = pd_ps.tile([BL, H], dt.float32, tag="tpo")
            for k in range(C):
                nc.tensor.matmul(out=ps_tpo[:], lhsT=tpT_bf[:, k * BL:(k + 1) * BL],
                                 rhs=wo1_sb[:, k, :], start=(k == 0), stop=(k == C - 1))
            tpo_bf = pd_w.tile([BL, H], dt.bfloat16)
            nc.vector.tensor_copy(tpo_bf[:], ps_tpo[:])
            tpo_rows = []
            for b in range(BL):
                tr_ = pd_w.tile([1, H], dt.bfloat16, name=f"tpo_row{b}")
                nc.sync.dma_start(tr_[:], tpo_bf[b:b + 1, :])
                tpo_rows.append(tr_)

            ps_tpoT = pd_ps.tile([128, C * BL], dt.float32, tag="small")
            for m in range(C):
                for k in range(C):
                    nc.tensor.matmul(
                        out=ps_tpoT[:, m * BL:(m + 1) * BL],
                        lhsT=wo1_sb[:, k, m * 128:(m + 1) * 128],
                        rhs=tpT_bf[:, k * BL:(k + 1) * BL],
                        start=(k == 0), stop=(k == C - 1))
            fb = pd_w.tile([128, C * BL], dt.float32)
            nc.vector.scalar_tensor_tensor(
                out=fb[:].rearrange("p (c b) -> p c b", c=C),
                in0=ps_tpoT[:].rearrange("p (c b) -> p c b", c=C),
                scalar=tbv_sb[:],
                in1=bo_sb[:][:, :, None].to_broadcast([128, C, BL]),
                op0=ALU.mult, op1=ALU.add)

            # hidden output
            hcat = pd_w.tile([128, 8 * BL], dt.bfloat16)
            for k in range(8):
                if k < C:
                    src = rsF[nw - 1][:, k, WS - 1, :]
                else:
                    src = rsB[nw - 1][:, k - C, 0, :]
                nc.sync.dma_start_transpose(hcat[:, k * BL:(k + 1) * BL], src)
            ps_hid = pd_ps.tile([128, C * BL], dt.float32, tag="small")
            for m in range(C):
                for k in range(8):
                    nc.tensor.matmul(
                        out=ps_hid[:, m * BL:(m + 1) * BL],
                        lhsT=who_sb[:, k, m * 128:(m + 1) * 128],
                        rhs=hcat[:, k * BL:(k + 1) * BL],
                        start=(k == 0), stop=(k == 7))
            hid_sb = pd_w.tile([128, C * BL], dt.float32)
            nc.vector.tensor_copy(hid_sb[:], ps_hid[:])
            for c in range(C):
                nc.sync.dma_start(
                    hidT[:, c, :].rearrange("b p -> p b"),
                    hid_sb[:, c * BL:(c + 1) * BL])

            # ---- main per-batch attention loop
            for b in range(BL):
                ew1 = pd_ew.tile([128, 8, s_steps], dt.bfloat16, tag="ew1")
                for di, d_is_f in ((0, True), (1, False)):
                    for w in range(nw):
                        rsw = rsF[w] if d_is_f else rsB[nw - 1 - w]
                        raw = pd.tile([WS, C * 128], dt.bfloat16, tag="ewraw")
                        nc.scalar.dma_start(
                            raw[:].rearrange("s (c f) -> s c f", c=C),
                            rsw[b, :, :, :].rearrange("c s f -> s c f"))
                        pst = pd_ps.tile([128, C * WS], dt.bfloat16, tag="ewps")
                        for c in range(C):
                            nc.tensor.transpose(
                                out=pst[:, c * WS:(c + 1) * WS],
                                in_=raw[:, c * 128:(c + 1) * 128],
                                identity=identWS[:])
                        for c in range(C):
                            nc.vector.tensor_copy(
                                ew1[:, di * C + c, w * WS:(w + 1) * WS],
                                pst[:, c * WS:(c + 1) * WS])

                ps_b1 = [pd_ps2.tile([128, s_steps], dt.float32, name=f"psb1{m}", tag=f"b1_{m}")
                         for m in range(C)]
                for m in range(C):
                    for k in range(8):
                        nc.tensor.matmul(
                            out=ps_b1[m][:],
                            lhsT=ww_sb[:, k, m * 128:(m + 1) * 128],
                            rhs=ew1[:, k, :],
                            start=(k == 0), stop=(k == 7))

                ps_ssum = pd_ps.tile([1, s_steps], dt.float32, tag="tpo")
                for t in range(T):
                    for c in range(C):
                        th_t = pd.tile([128, s_steps], dt.bfloat16, tag="tanh")
                        nc.scalar.activation(
                            th_t[:], ps_b1[c][:], AF.Tanh,
                            bias=bias2[:, (c * BL + b) * T + t:(c * BL + b) * T + t + 1])
                        nc.tensor.matmul(
                            out=ps_ssum[:], lhsT=wv_sb[:, c:c + 1], rhs=th_t[:],
                            start=(t == 0 and c == 0), stop=(t == T - 1 and c == C - 1))
                ssum_bf = pd.tile([1, s_steps], dt.bfloat16, tag="ssbf")
                nc.vector.tensor_copy(ssum_bf[:], ps_ssum[:])

                for m in range(C):
                    ps_o = pd_ps2.tile([128, s_steps], dt.float32, tag=f"b1_{m}")
                    for k in range(8):
                        nc.tensor.matmul(
                            out=ps_o[:],
                            lhsT=wo2_sb[:, k, m * 128:(m + 1) * 128],
                            rhs=ew1[:, k, :],
                            start=(k == 0), stop=False)
                    nc.tensor.matmul(
                        out=ps_o[:], lhsT=tpo_rows[b][:, m * 128:(m + 1) * 128],
                        rhs=ssum_bf[:], start=False, stop=True)
                    outc = pd.tile([128, s_steps], dt.float32, tag="outc")
                    nc.scalar.activation(outc[:], ps_o[:], AF.Identity,
                                         bias=fb[:, m * BL + b:m * BL + b + 1])
                    nc.sync.dma_start(enc_outT[b, m, :, :], outc[:])

        _scopeD.__exit__(None, None, None)

    return nc


# ---------------------------------------------------------------------------
# host side
# ---------------------------------------------------------------------------
_cache = {}


def _prep_in_maps(inputs, v=V, s_steps=S):
    f32 = np.float32
    bf16 = ml_dtypes.bfloat16
    i32 = np.int32

    def g(name):
        return np.asarray(inputs[name])

    src = g("src").astype(np.int64)
    theme = g("theme").astype(np.int64)
    keyword = g("keyword").astype(np.int64)
    tloc = s_steps * B // NC_
    nt = max(tloc // 128, 1)

    order_f = src.reshape(s_steps * B)                       # (s, b) ascending
    order_b = src[::-1, :].reshape(s_steps * B)              # s descending

    Wih = {"f": g("Wih_f"), "b": g("Wih_b")}
    Whh = {"f": g("Whh_f"), "b": g("Whh_b")}
    bih = {"f": g("bih_f"), "b": g("bih_b")}
    bhh = {"f": g("bhh_f"), "b": g("bhh_b")}

    def gxbias(d):
        # m-tiles: 0-3 r (bih+bhh), 4-7 z (bih+bhh), 8-11 n (bih only)
        bb = np.empty((12, 128), f32)
        full = bih[d] + bhh[d]
        for m in range(12):
            lo = m * 128
            if m >= 8:
                bb[m] = bih[d][lo:lo + 128]
            else:
                bb[m] = full[lo:lo + 128]
        return bb.T.copy()  # (128, 12)

    def bbnb(d):
        # (128, c*32+b) = bhh_n[c*128+p]
        bn = bhh[d][2 * H:3 * H].reshape(C, 128)  # [c, p]
        return np.repeat(bn.T[:, :, None], B, axis=2).reshape(128, C * B).copy()

    Ww, bw = g("Ww"), g("bw")
    Wt, bt = g("Wt"), g("bt")
    Wk, bk = g("Wk"), g("bk")
    wv, bv = g("wv"), g("bv")
    Wo, bo = g("Wo"), g("bo")
    Who = g("Who")

    wwT = np.ascontiguousarray(Ww.T).astype(bf16)       # (1024, 512)
    wo1T = np.ascontiguousarray(Wo[:, :H].T).astype(bf16)
    wo2T = np.ascontiguousarray(Wo[:, H:].T).astype(bf16)
    wtT = np.ascontiguousarray(Wt.T).astype(bf16)
    wkT = np.ascontiguousarray(Wk.T).astype(bf16)
    whoT = np.ascontiguousarray(Who.T).astype(bf16)
    wv_c = np.ascontiguousarray(wv.reshape(C, 128).T).astype(bf16)
    btT = np.ascontiguousarray(bt.reshape(C, 128).T).astype(f32)
    bkwT = np.ascontiguousarray((bk + bw).reshape(C, 128).T).astype(f32)
    boT = np.ascontiguousarray(bo.reshape(C, 128).T).astype(f32)
    tbv = np.full((128, 1), float(T) * float(bv), f32)

    src_tab = np.ascontiguousarray(g("src_tab")).astype(f32)
    theme_tab = np.ascontiguousarray(g("theme_tab")).astype(f32)
    keyword_tab = np.ascontiguousarray(g("keyword_tab")).astype(f32)

    zeros_w = np.zeros((H, G3), bf16)
    in_maps = []
    for k in range(NC_):
        im = {
            "src_tab": src_tab, "theme_tab": theme_tab, "keyword_tab": keyword_tab,
            "ids_f": np.ascontiguousarray(
                order_f[k * tloc:(k + 1) * tloc].reshape(nt, 128).T).astype(i32),
            "ids_b": np.ascontiguousarray(
                order_b[k * tloc:(k + 1) * tloc].reshape(nt, 128).T).astype(i32),
            "wihT_f": np.ascontiguousarray(Wih["f"].T).astype(bf16),
            "wihT_b": np.ascontiguousarray(Wih["b"].T).astype(bf16),
            "gxbias_f": gxbias("f"), "gxbias_b": gxbias("b"),
            "wwT": wwT, "wo1T": wo1T, "wo2T": wo2T, "wtT": wtT, "wkT": wkT,
            "whoT": whoT, "wv_c": wv_c, "btT": btT, "bkwT": bkwT, "boT": boT,
            "tbv": tbv,
            "theme_ids": theme[0, k * BL:(k + 1) * BL].reshape(BL, 1).astype(i32),
            "kw_ids": np.ascontiguousarray(
                keyword[:, k * BL:(k + 1) * BL].T.reshape(BL * T, 1)).astype(i32),
        }
        if k == 0:
            im["whhT"] = np.ascontiguousarray(Whh["f"].T).astype(bf16)
            im["bbnb"] = bbnb("f")
            im["mF"] = np.ones((128, 1), f32)
            im["mB"] = np.zeros((128, 1), f32)
        elif k == 1:
            im["whhT"] = np.ascontiguousarray(Whh["b"].T).astype(bf16)
            im["bbnb"] = bbnb("b")
            im["mF"] = np.zeros((128, 1), f32)
            im["mB"] = np.ones((128, 1), f32)
        else:
            im["whhT"] = zeros_w
            im["bbnb"] = np.zeros((128, 128), f32)
            im["mF"] = np.zeros((128, 1), f32)
            im["mB"] = np.zeros((128, 1), f32)
        in_maps.append(im)
    return in_maps


def _assemble(results, s_steps=S):
    enc = np.concatenate([r["enc_outT"] for r in results], axis=0)  # (32,4,128,S)
    enc_out = np.ascontiguousarray(enc.reshape(B, H, s_steps).transpose(2, 0, 1))
    hid = np.concatenate([r["hidT"] for r in results], axis=0)      # (32,4,128)
    hid_out = np.ascontiguousarray(hid.reshape(1, B, H))
    return enc_out.astype(np.float32), hid_out.astype(np.float32)


TRACE = False
LAST_RESULT = None


def _install_ntff_hook():
    import types as _types
    try:
        import antenv
        if not hasattr(antenv, "axon_hooks") and "antenv.axon_hooks" not in sys.modules:
            m = _types.ModuleType("antenv.axon_hooks")
            m._hook = None
            m.set_axon_ntff_profile_hook = lambda h, _m=m: setattr(_m, "_hook", h)
            m.get_axon_ntff_profile_hook = lambda _m=m: _m._hook
            sys.modules["antenv.axon_hooks"] = m
            antenv.axon_hooks = m
        from trn_agent_boot.trn_boot import _ntff_profile_via_ctypes
        h = _ntff_profile_via_ctypes("/opt/axon/libaxon_pjrt.so")
        if h is not None:
            sys.modules["antenv.axon_hooks"].set_axon_ntff_profile_hook(h)
    except Exception:
        pass


def kernel(**inputs):
    global LAST_RESULT
    key = "full"
    if key not in _cache:
        _cache[key] = _build_nc()
    nc = _cache[key]
    in_maps = _prep_in_maps(inputs)
    if TRACE:
        _install_ntff_hook()
    res = run_bass_kernel_spmd(nc, in_maps, list(range(NC_)), trace=TRACE)
    LAST_RESULT = res
    return _assemble(res.results)


# revision 28
# speedup vs baseline: 1.0582x; 1.0582x over previous
"""Trainium2 Bass kernel for nn_Encoder_77627238908751.

Strategy (8 NeuronCores, SPMD single program, role differences carried
entirely by per-core input DATA — no rank branching):

- Phase B (all cores): each core gathers 1/8 of the src embeddings for BOTH
  sequence orders (forward s-ascending and backward s-descending), PE-transposes
  them to feature-major, computes its 1/8 slice of the GRU input-gate
  preactivations gx^T = Wih @ emb^T for both directions, and AllGathers the
  full gx^T tensors so cores 0/1 have their direction's gx locally.
- Phase C: the sequential GRU recurrence, fully unrolled.  Core 0 runs the
  forward direction, core 1 the backward one (its inputs are the reversed
  sequence, so the same program computes the backward pass); helper cores run
  the same instructions on zero weights.  Feature-major layout: per step
  48 (128x128)x(128x32) bf16 matmuls accumulate the 3 gate preactivations in
  PSUM, gate math on DVE/ACT in fp32, h kept as an fp32 master + bf16 stream
  copy.  Each step's h is PE-transposed and written (masked per core) into
  per-window DRAM buffers; each 32-step window is ReduceScattered across the
  8 cores during the recurrence, leaving every core exactly its 4-batch shard
  of the bidirectional outputs.
- Phase D (all cores): additive-attention + output projection for the core's
  4 batches.  Key algebraic simplifications: scores @ theme_proj is rank-1
  (enc_w_2[b,s,:] = (sum_t scores[b,s,t]) * theme_proj[b,:]), so the (B,S,T,H)
  tensor is never materialized — tanh tiles are reduced on the fly against wv
  in PSUM, and the enc_w_2 @ Wo1^T term becomes a rank-1 outer-product matmul
  fused into the Wo2 accumulation.

Host side only reorders/slices/casts inputs and reassembles outputs.
"""

import sys
import types

sys.path.insert(0, "/opt/trn_rl_repo")
sys.path.insert(0, "/root/.axon_site")

import numpy as np
import ml_dtypes
import orjson

import concourse.bass as bass
import concourse.tile as tile
import concourse.mybir as mybir
from concourse.bass_utils import run_bass_kernel_spmd
from concourse.masks import make_identity

dt = mybir.dt
AF = mybir.ActivationFunctionType
ALU = mybir.AluOpType

# ---------------------------------------------------------------------------
# workaround: this walrus build rejects >1 sync wait per instruction; split
# extra on_wait entries onto preceding single-wait EventSemaphore instructions.
# ---------------------------------------------------------------------------
_MAX_WAITS = 1


def _split_waits(mod: dict) -> bool:
    changed = False
    ctr = 0
    for f in mod.get("functions", []):
        for bb in f.get("blocks", []):
            out = []
            for ins in bb.get("instructions", []):
                si = ins.get("sync_info")
                waits = (si or {}).get("on_wait") or []
                if len(waits) > _MAX_WAITS:
                    changed = True
                    extra, keep = waits[:-_MAX_WAITS], waits[-_MAX_WAITS:]
                    for w in extra:
                        ctr += 1
                        out.append({
                            "debug": ins.get("debug", 0),
                            "engine": ins["engine"],
                            "ins": [],
                            "name": f"{ins['name']}_xw{ctr}",
                            "opcode": "EventSemaphore",
                            "outs": [],
                            "sync_info": {"on_update": [], "on_wait": [w]},
                        })
                    si["on_wait"] = keep
                out.append(ins)
            bb["instructions"] = out
    return changed


if not getattr(bass.Bass, "_waitfix_installed", False):
    _orig_to_json_bytes = bass.Bass.to_json_bytes

    def _patched_to_json_bytes(self) -> bytes:
        raw = _orig_to_json_bytes(self)
        mod = orjson.loads(raw)
        if _split_waits(mod):
            raw = orjson.dumps(mod)
        return raw

    bass.Bass.to_json_bytes = _patched_to_json_bytes
    bass.Bass._waitfix_installed = True

# ---------------------------------------------------------------------------
# problem constants
# ---------------------------------------------------------------------------
V, H, B, S, T = 32000, 512, 32, 256, 16
NC_ = 8            # cores
C = 4              # 128-feature chunks of H
G3 = 3 * H         # 1536
WS = 32            # ReduceScatter window (steps)
NW = S // WS       # windows
BL = B // NC_      # batches per core (4)
TLOC = S * B // NC_  # tokens per core per direction (1024)
NT = TLOC // 128     # token tiles per core (8)
NB = TLOC // 512     # 512-token blocks per core (2)
PW = 16            # gx prefetch half-window (steps)


def _build_nc(v=V, s_steps=S):
    """Build the single SPMD Bass program."""
    nw = s_steps // WS
    tloc = s_steps * B // NC_
    nt = max(tloc // 128, 1)
    nb = max(tloc // 512, 1)
    nbl = min(512, tloc)           # token block width for gx matmul
    pw = min(PW, WS, tloc // B)

    nc = bass.Bass()

    # ---- I/O ------------------------------------------------------------
    src_tab = nc.declare_dram_parameter("src_tab", [v, H], dt.float32, isOutput=False)
    theme_tab = nc.declare_dram_parameter("theme_tab", [v, H], dt.float32, isOutput=False)
    keyword_tab = nc.declare_dram_parameter("keyword_tab", [v, H], dt.float32, isOutput=False)
    ids_f = nc.declare_dram_parameter("ids_f", [128, nt], dt.int32, isOutput=False)
    ids_b = nc.declare_dram_parameter("ids_b", [128, nt], dt.int32, isOutput=False)
    wihT_f = nc.declare_dram_parameter("wihT_f", [H, G3], dt.bfloat16, isOutput=False)
    wihT_b = nc.declare_dram_parameter("wihT_b", [H, G3], dt.bfloat16, isOutput=False)
    whhT = nc.declare_dram_parameter("whhT", [H, G3], dt.bfloat16, isOutput=False)
    gxbias_f = nc.declare_dram_parameter("gxbias_f", [128, 12], dt.float32, isOutput=False)
    gxbias_b = nc.declare_dram_parameter("gxbias_b", [128, 12], dt.float32, isOutput=False)
    bbnb = nc.declare_dram_parameter("bbnb", [128, 128], dt.float32, isOutput=False)
    mF = nc.declare_dram_parameter("mF", [128, 1], dt.float32, isOutput=False)
    mB = nc.declare_dram_parameter("mB", [128, 1], dt.float32, isOutput=False)
    theme_ids = nc.declare_dram_parameter("theme_ids", [BL, 1], dt.int32, isOutput=False)
    kw_ids = nc.declare_dram_parameter("kw_ids", [BL * T, 1], dt.int32, isOutput=False)
    wwT = nc.declare_dram_parameter("wwT", [2 * H, H], dt.bfloat16, isOutput=False)
    wo1T = nc.declare_dram_parameter("wo1T", [H, H], dt.bfloat16, isOutput=False)
    wo2T = nc.declare_dram_parameter("wo2T", [2 * H, H], dt.bfloat16, isOutput=False)
    wtT = nc.declare_dram_parameter("wtT", [H, H], dt.bfloat16, isOutput=False)
    wkT = nc.declare_dram_parameter("wkT", [H, H], dt.bfloat16, isOutput=False)
    whoT = nc.declare_dram_parameter("whoT", [2 * H, H], dt.bfloat16, isOutput=False)
    wv_c = nc.declare_dram_parameter("wv_c", [128, C], dt.bfloat16, isOutput=False)
    btT = nc.declare_dram_parameter("btT", [128, C], dt.float32, isOutput=False)
    bkwT = nc.declare_dram_parameter("bkwT", [128, C], dt.float32, isOutput=False)
    boT = nc.declare_dram_parameter("boT", [128, C], dt.float32, isOutput=False)
    tbv = nc.declare_dram_parameter("tbv", [128, 1], dt.float32, isOutput=False)

    enc_outT = nc.declare_dram_parameter("enc_outT", [BL, C, 128, s_steps], dt.float32, isOutput=True)
    hidT = nc.declare_dram_parameter("hidT", [BL, C, 128], dt.float32, isOutput=True)

    rg = [list(range(NC_))]

    from contextlib import ExitStack

    with tile.TileContext(nc) as tc, ExitStack() as stack:
        # persistent pools
        const = stack.enter_context(tc.tile_pool(name="const", bufs=1))
        dram = stack.enter_context(tc.tile_pool(name="dram", bufs=1, space="DRAM"))

        ident = const.tile([128, 128], dt.float32)
        make_identity(nc, ident[:])
        identBL = const.tile([BL, BL], dt.float32)
        make_identity(nc, identBL[:])
        identKW = const.tile([BL * T, BL * T], dt.float32)
        make_identity(nc, identKW[:])
        identWS = const.tile([WS, WS], dt.bfloat16)
        make_identity(nc, identWS[:])

        mF_sb = const.tile([128, 1], dt.float32)
        nc.sync.dma_start(mF_sb[:], mF[:])
        mB_sb = const.tile([128, 1], dt.float32)
        nc.sync.dma_start(mB_sb[:], mB[:])
        identBF = const.tile([128, 128], dt.bfloat16)
        nc.vector.tensor_copy(identBF[:], ident[:])
        identF = const.tile([128, 128], dt.bfloat16)
        nc.vector.tensor_scalar_mul(identF[:], ident[:], mF_sb[:])
        identB = const.tile([128, 128], dt.bfloat16)
        nc.vector.tensor_scalar_mul(identB[:], ident[:], mB_sb[:])
        bbnb_sb = const.tile([128, 128], dt.float32)
        nc.sync.dma_start(bbnb_sb[:], bbnb[:])
        bbnb_bf = const.tile([128, 128], dt.bfloat16)
        nc.vector.tensor_copy(bbnb_bf[:], bbnb_sb[:])

        whh_sb = const.tile([128, C, G3], dt.bfloat16)
        nc.sync.dma_start(whh_sb[:], whhT[:].rearrange("(k p) g -> p k g", p=128))

        # DRAM intermediates
        gx_contrib = {}
        gx_full = {}
        for d in ("f", "b"):
            gx_contrib[d] = dram.tile([128, 12, tloc], dt.bfloat16, name=f"gxc{d}")
            gx_full[d] = dram.tile([NC_, 128, 12, tloc], dt.bfloat16, name=f"gxfull{d}", addr_space="Shared")
        bufF = [dram.tile([B, C, WS, 128], dt.bfloat16, name=f"bufF{w}") for w in range(nw)]
        bufB = [dram.tile([B, C, WS, 128], dt.bfloat16, name=f"bufB{w}") for w in range(nw)]
        rsF = [dram.tile([BL, C, WS, 128], dt.bfloat16, name=f"rsF{w}") for w in range(nw)]
        rsB = [dram.tile([BL, C, WS, 128], dt.bfloat16, name=f"rsB{w}") for w in range(nw)]

        # ---- Phase B: embedding gather + transpose + gx slices + AllGather
        _scopeB = nc.named_scope("phaseB"); _scopeB.__enter__()
        with (
            tc.tile_pool(name="pb_sb", bufs=2) as pb,
            tc.tile_pool(name="pb_semb", bufs=1) as pb_semb,
            tc.tile_pool(name="pb_wih", bufs=1) as pb_wih,
            tc.tile_pool(name="pb_ps", bufs=2, space="PSUM") as pb_ps,
        ):
            wih_sb = {}
            for d, w_in in (("f", wihT_f), ("b", wihT_b)):
                w_sb = pb_wih.tile([128, C, G3], dt.bfloat16, name=f"wih{d}", tag=f"wih{d}")
                nc.sync.dma_start(w_sb[:], w_in[:].rearrange("(k p) g -> p k g", p=128))
                wih_sb[d] = w_sb
            gxb_sb = {}
            for d, b_in in (("f", gxbias_f), ("b", gxbias_b)):
                t_ = const.tile([128, 12], dt.float32, name=f"gxb{d}", tag=f"gxb{d}")
                nc.sync.dma_start(t_[:], b_in[:])
                gxb_sb[d] = t_

            idt = {}
            for d, i_in in (("f", ids_f), ("b", ids_b)):
                t_ = const.tile([128, nt], dt.int32, name=f"idt{d}", tag=f"idt{d}")
                nc.sync.dma_start(t_[:], i_in[:])
                idt[d] = t_

            semb = {}
            for d in ("f", "b"):
                semb[d] = [pb_semb.tile([128, nt * 128], dt.bfloat16, name=f"semb{d}{c}", tag=f"semb{d}{c}")
                           for c in range(C)]
                for tt in range(nt):
                    g = pb.tile([128, H], dt.float32, tag="gath")
                    nc.gpsimd.indirect_dma_start(
                        out=g[:], out_offset=None, in_=src_tab[:, :],
                        in_offset=bass.IndirectOffsetOnAxis(ap=idt[d][:, tt:tt + 1], axis=0),
                    )
                    ps = pb_ps.tile([128, H], dt.float32, tag="tp")
                    for c in range(C):
                        nc.tensor.transpose(
                            out=ps[:, c * 128:(c + 1) * 128],
                            in_=g[:, c * 128:(c + 1) * 128],
                            identity=ident[:],
                        )
                    for c in range(C):
                        nc.vector.tensor_copy(
                            semb[d][c][:, tt * 128:(tt + 1) * 128],
                            ps[:, c * 128:(c + 1) * 128],
                        )

            for d in ("f", "b"):
                for m in range(12):
                    for q in range(tloc // nbl):
                        ps = pb_ps.tile([128, nbl], dt.float32, tag="gx")
                        for k in range(C):
                            nc.tensor.matmul(
                                out=ps[:],
                                lhsT=wih_sb[d][:, k, m * 128:(m + 1) * 128],
                                rhs=semb[d][k][:, q * nbl:(q + 1) * nbl],
                                start=(k == 0), stop=(k == C - 1),
                            )
                        gxs = pb.tile([128, nbl], dt.bfloat16, tag="gxs")
                        nc.scalar.activation(gxs[:], ps[:], AF.Identity,
                                             bias=gxb_sb[d][:, m:m + 1])
                        nc.sync.dma_start(
                            gx_contrib[d][:, m, q * nbl:(q + 1) * nbl], gxs[:])

            for d in ("f", "b"):
                nc.gpsimd.collective_compute(
                    "AllGather", ALU.bypass, replica_groups=rg,
                    ins=[gx_contrib[d].opt()], outs=[gx_full[d].opt()],
                )

        _scopeB.__exit__(None, None, None)
        # ---- Phase C: recurrence -----------------------------------------
        _scopeC = nc.named_scope("phaseC"); _scopeC.__enter__()
        with (
            tc.tile_pool(name="pc_h", bufs=2) as pc_h,
            tc.tile_pool(name="pc_g", bufs=2) as pc_g,
            tc.tile_pool(name="pc_win", bufs=2) as pc_win,
            tc.tile_pool(name="pc_ps", bufs=2, space="PSUM") as pc_ps,
            tc.tile_pool(name="pc_pst", bufs=1, space="PSUM") as pc_pst,
        ):
            hbf = pc_h.tile([128, 128], dt.bfloat16, tag="hbf")
            nc.gpsimd.memset(hbf[:], 0.0)

            # per-core AG slot: the recurrence consumes windows by rank-
            # uniform global step index; window w tokens live in slot
            # (w*WS*B)//tloc at offset (w*WS*B) % tloc.
            gxwin = None
            GATE = ((0, "r"), (2, "n"), (1, "z"))  # gate order; m-base = idx*4

            for t in range(s_steps):
                if t % pw == 0:
                    tok0 = t * B
                    slot = tok0 // tloc
                    off = tok0 % tloc
                    wid = pw * B
                    raws = {}
                    for d, eng in (("f", nc.sync), ("b", nc.scalar)):
                        rw = pc_win.tile([128, 12, wid], dt.bfloat16, tag=f"raw{d}")
                        eng.dma_start(rw[:], gx_full[d][slot, :, :, off:off + wid])
                        raws[d] = rw
                    gxwin = pc_win.tile([128, 12, wid], dt.bfloat16, tag="gxwin")
                    nc.vector.tensor_scalar_mul(gxwin[:], raws["b"][:], mB_sb[:])
                    nc.vector.scalar_tensor_tensor(
                        out=gxwin[:], in0=raws["f"][:], scalar=mF_sb[:],
                        in1=gxwin[:], op0=ALU.mult, op1=ALU.add)

                toff = (t % pw) * B
                ps_g = {}
                for gi, gname in GATE:
                    ps = pc_ps.tile([128, 128], dt.float32, tag=f"ps{gname}")
                    for c in range(C):
                        m = gi * 4 + c
                        for k in range(C):
                            nc.tensor.matmul(
                                out=ps[:, c * B:c * B + B],
                                lhsT=whh_sb[:, k, m * 128:(m + 1) * 128],
                                rhs=hbf[:, k * B:k * B + B],
                                start=(k == 0), stop=(k == C - 1),
                            )
                    ps_g[gname] = ps

                def gx_slice(gi):
                    return gxwin[:, gi * 4:(gi + 1) * 4, toff:toff + B]

                def ps3(ps):
                    return ps[:].rearrange("p (c b) -> p c b", c=C)

                rpre = pc_g.tile([128, 128], dt.float32, tag="rpre")
                nc.vector.tensor_add(ps3(rpre), ps3(ps_g["r"]), gx_slice(0))
                r = pc_g.tile([128, 128], dt.float32, tag="r")
                nc.scalar.activation(r[:], rpre[:], AF.Sigmoid)

                npre = pc_g.tile([128, 128], dt.float32, tag="npre")
                nc.vector.tensor_add(npre[:], ps_g["n"][:], bbnb_sb[:])
                nc.vector.tensor_mul(npre[:], r[:], npre[:])
                nc.vector.tensor_add(ps3(npre), ps3(npre), gx_slice(2))
                n = pc_g.tile([128, 128], dt.float32, tag="n")
                nc.scalar.activation(n[:], npre[:], AF.Tanh)

                s1 = pc_g.tile([128, 128], dt.float32, tag="s1")
                nc.vector.tensor_sub(s1[:], hbf[:], n[:])

                zpre = pc_g.tile([128, 128], dt.float32, tag="zpre")
                nc.vector.tensor_add(ps3(zpre), ps3(ps_g["z"]), gx_slice(1))
                z = pc_g.tile([128, 128], dt.float32, tag="z")
                nc.scalar.activation(z[:], zpre[:], AF.Sigmoid)

                tmp = pc_g.tile([128, 128], dt.float32, tag="s2")
                nc.vector.scalar_tensor_tensor(
                    out=tmp[:], in0=z[:], scalar=1.0, in1=s1[:],
                    op0=ALU.bypass, op1=ALU.mult)
                hbfn = pc_h.tile([128, 128], dt.bfloat16, tag="hbf")
                nc.vector.tensor_add(hbfn[:], tmp[:], n[:])
                hbf = hbfn

                # masked transposes straight to PSUM, DMA out from PSUM
                pstF = pc_pst.tile([128, 128], dt.bfloat16, tag="trF")
                nc.tensor.transpose(out=pstF[:], in_=hbf[:], identity=identF[:])
                pstB = pc_pst.tile([128, 128], dt.bfloat16, tag="trB")
                nc.tensor.transpose(out=pstB[:], in_=hbf[:], identity=identB[:])

                stF = pc_g.tile([128, 128], dt.bfloat16, tag="stF")
                nc.scalar.activation(stF[:], pstF[:], AF.Identity)
                stB = pc_g.tile([128, 128], dt.bfloat16, tag="stB")
                nc.scalar.activation(stB[:], pstB[:], AF.Identity)
                w = t // WS
                tl = t - w * WS
                dstF = bufF[w][:, :, tl, :].rearrange("b c f -> c b f")
                nc.sync.dma_start(dstF, stF[:])
                tlb = WS - 1 - tl
                dstB = bufB[w][:, :, tlb, :].rearrange("b c f -> c b f")
                nc.sync.dma_start(dstB, stB[:])

                if tl == WS - 1:
                    nc.gpsimd.collective_compute(
                        "ReduceScatter", ALU.add, replica_groups=rg,
                        ins=[bufF[w].opt()], outs=[rsF[w].opt()])
                    nc.gpsimd.collective_compute(
                        "ReduceScatter", ALU.add, replica_groups=rg,
                        ins=[bufB[w].opt()], outs=[rsB[w].opt()])

        _scopeC.__exit__(None, None, None)
        # ---- Phase D: attention + outputs --------------------------------
        _scopeD = nc.named_scope("phaseD"); _scopeD.__enter__()
        with (
            tc.tile_pool(name="pd_w", bufs=1) as pd_w,
            tc.tile_pool(name="pd_sb", bufs=2) as pd,
            tc.tile_pool(name="pd_ew", bufs=2) as pd_ew,
            tc.tile_pool(name="pd_ps", bufs=1, space="PSUM") as pd_ps,
            tc.tile_pool(name="pd_ps2", bufs=1, space="PSUM") as pd_ps2,
        ):
            ww_sb = pd_w.tile([128, 8, H], dt.bfloat16)
            nc.sync.dma_start(ww_sb[:], wwT[:].rearrange("(k p) g -> p k g", p=128))
            wo2_sb = pd_w.tile([128, 8, H], dt.bfloat16)
            nc.sync.dma_start(wo2_sb[:], wo2T[:].rearrange("(k p) g -> p k g", p=128))
            wo1_sb = pd_w.tile([128, C, H], dt.bfloat16)
            nc.sync.dma_start(wo1_sb[:], wo1T[:].rearrange("(k p) g -> p k g", p=128))
            wt_sb = pd_w.tile([128, C, H], dt.bfloat16)
            nc.sync.dma_start(wt_sb[:], wtT[:].rearrange("(k p) g -> p k g", p=128))
            wk_sb = pd_w.tile([128, C, H], dt.bfloat16)
            nc.sync.dma_start(wk_sb[:], wkT[:].rearrange("(k p) g -> p k g", p=128))
            who_sb = pd_w.tile([128, 8, H], dt.bfloat16)
            nc.sync.dma_start(who_sb[:], whoT[:].rearrange("(k p) g -> p k g", p=128))
            wv_sb = pd_w.tile([128, C], dt.bfloat16)
            nc.sync.dma_start(wv_sb[:], wv_c[:])
            bt_sb = pd_w.tile([128, C], dt.float32)
            nc.sync.dma_start(bt_sb[:], btT[:])
            bkw_sb = pd_w.tile([128, C], dt.float32)
            nc.sync.dma_start(bkw_sb[:], bkwT[:])
            bo_sb = pd_w.tile([128, C], dt.float32)
            nc.sync.dma_start(bo_sb[:], boT[:])
            tbv_sb = pd_w.tile([128, 1], dt.float32)
            nc.sync.dma_start(tbv_sb[:], tbv[:])

        # theme / keyword embeddings + projections (per-core b-shard)
            tid = pd_w.tile([BL, 1], dt.int32)
            nc.sync.dma_start(tid[:], theme_ids[:])
            kid = pd_w.tile([BL * T, 1], dt.int32)
            nc.sync.dma_start(kid[:], kw_ids[:])
            th_emb = pd_w.tile([BL, H], dt.float32)
            nc.gpsimd.indirect_dma_start(
                out=th_emb[:], out_offset=None, in_=theme_tab[:, :],
                in_offset=bass.IndirectOffsetOnAxis(ap=tid[:, :1], axis=0))
            kw_emb = pd_w.tile([BL * T, H], dt.float32)
            nc.gpsimd.indirect_dma_start(
                out=kw_emb[:], out_offset=None, in_=keyword_tab[:, :],
                in_offset=bass.IndirectOffsetOnAxis(ap=kid[:, :1], axis=0))

            ps_th = pd_ps.tile([128, C * BL], dt.float32, tag="small")
            for c in range(C):
                nc.tensor.transpose(out=ps_th[:, c * BL:(c + 1) * BL],
                                    in_=th_emb[:, c * 128:(c + 1) * 128],
                                    identity=identBL[:])
            thT = pd_w.tile([128, C * BL], dt.bfloat16)
            nc.vector.tensor_copy(thT[:], ps_th[:])

            ps_kw = pd_ps.tile([128, C * BL * T], dt.float32, tag="kw")
            for c in range(C):
                nc.tensor.transpose(out=ps_kw[:, c * BL * T:(c + 1) * BL * T],
                                    in_=kw_emb[:, c * 128:(c + 1) * 128],
                                    identity=identKW[:])
            kwT = pd_w.tile([128, C * BL * T], dt.bfloat16)
            nc.vector.tensor_copy(kwT[:], ps_kw[:])

            # tpT (feature-major theme projection, fp32 + bf16)
            ps_tp = pd_ps.tile([128, C * BL], dt.float32, tag="small")
            for m in range(C):
                for k in range(C):
                    nc.tensor.matmul(
                        out=ps_tp[:, m * BL:(m + 1) * BL],
                        lhsT=wt_sb[:, k, m * 128:(m + 1) * 128],
                        rhs=thT[:, k * BL:(k + 1) * BL],
                        start=(k == 0), stop=(k == C - 1))
            tpT = pd_w.tile([128, C * BL], dt.float32)
            for m in range(C):
                nc.scalar.activation(tpT[:, m * BL:(m + 1) * BL],
                                     ps_tp[:, m * BL:(m + 1) * BL],
                                     AF.Identity, bias=bt_sb[:, m:m + 1])
            tpT_bf = pd_w.tile([128, C * BL], dt.bfloat16)
            nc.vector.tensor_copy(tpT_bf[:], tpT[:])

            # b3T + bias2
            ps_b3 = pd_ps.tile([128, C * BL * T], dt.float32, tag="kw")
            for m in range(C):
                for k in range(C):
                    nc.tensor.matmul(
                        out=ps_b3[:, m * BL * T:(m + 1) * BL * T],
                        lhsT=wk_sb[:, k, m * 128:(m + 1) * 128],
                        rhs=kwT[:, k * BL * T:(k + 1) * BL * T],
                        start=(k == 0), stop=(k == C - 1))
            b3_sb = pd_w.tile([128, C * BL * T], dt.float32)
            for m in range(C):
                nc.scalar.activation(b3_sb[:, m * BL * T:(m + 1) * BL * T],
                                     ps_b3[:, m * BL * T:(m + 1) * BL * T],
                                     AF.Identity, bias=bkw_sb[:, m:m + 1])
            bias2 = pd_w.tile([128, C * BL * T], dt.float32)
            nc.vector.tensor_add(
                bias2[:].rearrange("p (c b t) -> p c b t", c=C, b=BL),
                b3_sb[:].rearrange("p (c b t) -> p c b t", c=C, b=BL),
                tpT[:].rearrange("p (c b) -> p c b", c=C)[:, :, :, None].to_broadcast([128, C, BL, T]),
            )

            # tpo row-major + feature-major + final bias
            ps_tpo = pd_ps.tile([BL, H], dt.float32, tag="tpo")
            for k in range(C):
                nc.tensor.matmul(out=ps_tpo[:], lhsT=tpT_bf[:, k * BL:(k + 1) * BL],
                                 rhs=wo1_sb[:, k, :], start=(k == 0), stop=(k == C - 1))
            tpo_bf = pd_w.tile([BL, H], dt.bfloat16)
            nc.vector.tensor_copy(tpo_bf[:], ps_tpo[:])
            tpo_rows = []
            for b in range(BL):
                tr_ = pd_w.tile([1, H], dt.bfloat16, name=f"tpo_row{b}")
                nc.sync.dma_start(tr_[:], tpo_bf[b:b + 1, :])
                tpo_rows.append(tr_)

            ps_tpoT = pd_ps.tile([128, C * BL], dt.float32, tag="small")
            for m in range(C):
                for k in range(C):
                    nc.tensor.matmul(
                        out=ps_tpoT[:, m * BL:(m + 1) * BL],
                        lhsT=wo1_sb[:, k, m * 128:(m + 1) * 128],
                        rhs=tpT_bf[:, k * BL:(k + 1) * BL],
                        start=(k == 0), stop=(k == C - 1))
            fb = pd_w.tile([128, C * BL], dt.float32)
            nc.vector.scalar_tensor_tensor(
                out=fb[:].rearrange("p (c b) -> p c b", c=C),
                in0=ps_tpoT[:].rearrange("p (c b) -> p c b", c=C),
                scalar=tbv_sb[:],
                in1=bo_sb[:][:, :, None].to_broadcast([128, C, BL]),
                op0=ALU.mult, op1=ALU.add)

            # hidden output
            hcat = pd_w.tile([128, 8 * BL], dt.bfloat16)
            for k in range(8):
                if k < C:
                    src = rsF[nw - 1][:, k, WS - 1, :]
                else:
                    src = rsB[nw - 1][:, k - C, 0, :]
                nc.sync.dma_start_transpose(hcat[:, k * BL:(k + 1) * BL], src)
            ps_hid = pd_ps.tile([128, C * BL], dt.float32, tag="small")
            for m in range(C):
                for k in range(8):
                    nc.tensor.matmul(
                        out=ps_hid[:, m * BL:(m + 1) * BL],
                        lhsT=who_sb[:, k, m * 128:(m + 1) * 128],
                        rhs=hcat[:, k * BL:(k + 1) * BL],
                        start=(k == 0), stop=(k == 7))
            hid_sb = pd_w.tile([128, C * BL], dt.float32)
            nc.vector.tensor_copy(hid_sb[:], ps_hid[:])
            for c in range(C):
                nc.sync.dma_start(
                    hidT[:, c, :].rearrange("b p -> p b"),
                    hid_sb[:, c * BL:(c + 1) * BL])

            # ---- main per-batch attention loop
            for b in range(BL):
                ew1 = pd_ew.tile([128, 8, s_steps], dt.bfloat16, tag="ew1")
                for di, d_is_f in ((0, True), (1, False)):
                    for w in range(nw):
                        rsw = rsF[w] if d_is_f else rsB[nw - 1 - w]
                        raw = pd.tile([WS, C * 128], dt.bfloat16, tag="ewraw", bufs=3)
                        nc.scalar.dma_start(
                            raw[:].rearrange("s (c f) -> s c f", c=C),
                            rsw[b, :, :, :].rearrange("c s f -> s c f"))
                        pst = pd_ps.tile([128, C * WS], dt.bfloat16, tag="ewps")
                        for c in range(C):
                            nc.tensor.transpose(
                                out=pst[:, c * WS:(c + 1) * WS],
                                in_=raw[:, c * 128:(c + 1) * 128],
                                identity=identWS[:])
                        for c in range(C):
                            nc.vector.tensor_copy(
                                ew1[:, di * C + c, w * WS:(w + 1) * WS],
                                pst[:, c * WS:(c + 1) * WS])

                ps_b1 = [pd_ps2.tile([128, s_steps], dt.float32, name=f"psb1{m}", tag=f"b1_{m}")
                         for m in range(C)]
                for m in range(C):
                    for k in range(8):
                        nc.tensor.matmul(
                            out=ps_b1[m][:],
                            lhsT=ww_sb[:, k, m * 128:(m + 1) * 128],
                            rhs=ew1[:, k, :],
                            start=(k == 0), stop=(k == 7))

                ps_ssum = pd_ps.tile([1, s_steps], dt.float32, tag="tpo")
                for t in range(T):
                    for c in range(C):
                        th_t = pd.tile([128, s_steps], dt.bfloat16, tag="tanh", bufs=3)
                        nc.scalar.activation(
                            th_t[:], ps_b1[c][:], AF.Tanh,
                            bias=bias2[:, (c * BL + b) * T + t:(c * BL + b) * T + t + 1])
                        nc.tensor.matmul(
                            out=ps_ssum[:], lhsT=wv_sb[:, c:c + 1], rhs=th_t[:],
                            start=(t == 0 and c == 0), stop=(t == T - 1 and c == C - 1))
                ssum_bf = pd.tile([1, s_steps], dt.bfloat16, tag="ssbf")
                nc.vector.tensor_copy(ssum_bf[:], ps_ssum[:])

                for m in range(C):
                    ps_o = pd_ps2.tile([128, s_steps], dt.float32, tag=f"b1_{m}")
                    for k in range(8):
                        nc.tensor.matmul(
                            out=ps_o[:],
                            lhsT=wo2_sb[:, k, m * 128:(m + 1) * 128],
                            rhs=ew1[:, k, :],
                            start=(k == 0), stop=False)
                    nc.tensor.matmul(
                        out=ps_o[:], lhsT=tpo_rows[b][:, m * 128:(m + 1) * 128],
                        rhs=ssum_bf[:], start=False, stop=True)
                    outc = pd.tile([128, s_steps], dt.float32, tag="outc")
                    nc.scalar.activation(outc[:], ps_o[:], AF.Identity,
                                         bias=fb[:, m * BL + b:m * BL + b + 1])
                    nc.sync.dma_start(enc_outT[b, m, :, :], outc[:])

        _scopeD.__exit__(None, None, None)

    return nc


# ---------------------------------------------------------------------------
# host side
# ---------------------------------------------------------------------------
_cache = {}


def _prep_in_maps(inputs, v=V, s_steps=S):
    f32 = np.float32
    bf16 = ml_dtypes.bfloat16
    i32 = np.int32

    def g(name):
        return np.asarray(inputs[name])

    src = g("src").astype(np.int64)
    theme = g("theme").astype(np.int64)
    keyword = g("keyword").astype(np.int64)
    tloc = s_steps * B // NC_
    nt = max(tloc // 128, 1)

    order_f = src.reshape(s_steps * B)                       # (s, b) ascending
    order_b = src[::-1, :].reshape(s_steps * B)              # s descending

    Wih = {"f": g("Wih_f"), "b": g("Wih_b")}
    Whh = {"f": g("Whh_f"), "b": g("Whh_b")}
    bih = {"f": g("bih_f"), "b": g("bih_b")}
    bhh = {"f": g("bhh_f"), "b": g("bhh_b")}

    def gxbias(d):
        # m-tiles: 0-3 r (bih+bhh), 4-7 z (bih+bhh), 8-11 n (bih only)
        bb = np.empty((12, 128), f32)
        full = bih[d] + bhh[d]
        for m in range(12):
            lo = m * 128
            if m >= 8:
                bb[m] = bih[d][lo:lo + 128]
            else:
                bb[m] = full[lo:lo + 128]
        return bb.T.copy()  # (128, 12)

    def bbnb(d):
        # (128, c*32+b) = bhh_n[c*128+p]
        bn = bhh[d][2 * H:3 * H].reshape(C, 128)  # [c, p]
        return np.repeat(bn.T[:, :, None], B, axis=2).reshape(128, C * B).copy()

    Ww, bw = g("Ww"), g("bw")
    Wt, bt = g("Wt"), g("bt")
    Wk, bk = g("Wk"), g("bk")
    wv, bv = g("wv"), g("bv")
    Wo, bo = g("Wo"), g("bo")
    Who = g("Who")

    wwT = np.ascontiguousarray(Ww.T).astype(bf16)       # (1024, 512)
    wo1T = np.ascontiguousarray(Wo[:, :H].T).astype(bf16)
    wo2T = np.ascontiguousarray(Wo[:, H:].T).astype(bf16)
    wtT = np.ascontiguousarray(Wt.T).astype(bf16)
    wkT = np.ascontiguousarray(Wk.T).astype(bf16)
    whoT = np.ascontiguousarray(Who.T).astype(bf16)
    wv_c = np.ascontiguousarray(wv.reshape(C, 128).T).astype(bf16)
    btT = np.ascontiguousarray(bt.reshape(C, 128).T).astype(f32)
    bkwT = np.ascontiguousarray((bk + bw).reshape(C, 128).T).astype(f32)
    boT = np.ascontiguousarray(bo.reshape(C, 128).T).astype(f32)
    tbv = np.full((128, 1), float(T) * float(bv), f32)

    src_tab = np.ascontiguousarray(g("src_tab")).astype(f32)
    theme_tab = np.ascontiguousarray(g("theme_tab")).astype(f32)
    keyword_tab = np.ascontiguousarray(g("keyword_tab")).astype(f32)

    zeros_w = np.zeros((H, G3), bf16)
    in_maps = []
    for k in range(NC_):
        im = {
            "src_tab": src_tab, "theme_tab": theme_tab, "keyword_tab": keyword_tab,
            "ids_f": np.ascontiguousarray(
                order_f[k * tloc:(k + 1) * tloc].reshape(nt, 128).T).astype(i32),
            "ids_b": np.ascontiguousarray(
                order_b[k * tloc:(k + 1) * tloc].reshape(nt, 128).T).astype(i32),
            "wihT_f": np.ascontiguousarray(Wih["f"].T).astype(bf16),
            "wihT_b": np.ascontiguousarray(Wih["b"].T).astype(bf16),
            "gxbias_f": gxbias("f"), "gxbias_b": gxbias("b"),
            "wwT": wwT, "wo1T": wo1T, "wo2T": wo2T, "wtT": wtT, "wkT": wkT,
            "whoT": whoT, "wv_c": wv_c, "btT": btT, "bkwT": bkwT, "boT": boT,
            "tbv": tbv,
            "theme_ids": theme[0, k * BL:(k + 1) * BL].reshape(BL, 1).astype(i32),
            "kw_ids": np.ascontiguousarray(
                keyword[:, k * BL:(k + 1) * BL].T.reshape(BL * T, 1)).astype(i32),
        }
        if k == 0:
            im["whhT"] = np.ascontiguousarray(Whh["f"].T).astype(bf16)
            im["bbnb"] = bbnb("f")
            im["mF"] = np.ones((128, 1), f32)
            im["mB"] = np.zeros((128, 1), f32)
        elif k == 1:
            im["whhT"] = np.ascontiguousarray(Whh["b"].T).astype(bf16)
            im["bbnb"] = bbnb("b")
            im["mF"] = np.zeros((128, 1), f32)
            im["mB"] = np.ones((128, 1), f32)
        else:
            im["whhT"] = zeros_w
            im["bbnb"] = np.zeros((128, 128), f32)
            im["mF"] = np.zeros((128, 1), f32)
            im["mB"] = np.zeros((128, 1), f32)
        in_maps.append(im)
    return in_maps


def _assemble(results, s_steps=S):
    enc = np.concatenate([r["enc_outT"] for r in results], axis=0)  # (32,4,128,S)
    enc_out = np.ascontiguousarray(enc.reshape(B, H, s_steps).transpose(2, 0, 1))
    hid = np.concatenate([r["hidT"] for r in results], axis=0)      # (32,4,128)
    hid_out = np.ascontiguousarray(hid.reshape(1, B, H))
    return enc_out.astype(np.float32), hid_out.astype(np.float32)


TRACE = False
LAST_RESULT = None


def _install_ntff_hook():
    import types as _types
    try:
        import antenv
        if not hasattr(antenv, "axon_hooks") and "antenv.axon_hooks" not in sys.modules:
            m = _types.ModuleType("antenv.axon_hooks")
            m._hook = None
            m.set_axon_ntff_profile_hook = lambda h, _m=m: setattr(_m, "_hook", h)
            m.get_axon_ntff_profile_hook = lambda _m=m: _m._hook
            sys.modules["antenv.axon_hooks"] = m
            antenv.axon_hooks = m
        from trn_agent_boot.trn_boot import _ntff_profile_via_ctypes
        h = _ntff_profile_via_ctypes("/opt/axon/libaxon_pjrt.so")
        if h is not None:
            sys.modules["antenv.axon_hooks"].set_axon_ntff_profile_hook(h)
    except Exception:
        pass


def kernel(**inputs):
    global LAST_RESULT
    key = "full"
    if key not in _cache:
        _cache[key] = _build_nc()
    nc = _cache[key]
    in_maps = _prep_in_maps(inputs)
    if TRACE:
        _install_ntff_hook()
    res = run_bass_kernel_spmd(nc, in_maps, list(range(NC_)), trace=TRACE)
    LAST_RESULT = res
    return _assemble(res.results)
